# revision 4
# baseline (speedup 1.0000x reference)
"""Bahdanau attention on 8 Trainium2 NeuronCores (Bass/Tile).

Data-parallel over batch: B=64 -> 8 rows per core; weights replicated.

Per-core dataflow (BL=8 batch rows, S=2048, H=512):
  keysT  : host-pretransposed keys slice [BL, H, S] (so the U-projection
           matmul can contract over H on the partition dim, streamed fp32r)
  UkT    = U_w @ keys_b^T           PE, fp32r, PSUM [g=128, s=512] tiles
  energy = tanh(UkT + bias_b[g])    ACT, bias = (W_w q_b + W_b + U_b)[g]
  logits = v^T energy               PE, v replicated over all 128 out rows
  softmax over s                    DVE max (negated) + ACT exp w/ accum Z
  context= (exp . keysT) / Z        DVE fused multiply+reduce over s
  attw   = exp / Z                  row 0 of the replicated exp tiles
"""

import sys
from contextlib import ExitStack

import numpy as np

try:
    import concourse.bass as bass  # noqa: F401
except ImportError:  # pragma: no cover
    sys.path.insert(0, "/opt/trn_rl_repo")

import concourse.bacc as bacc
import concourse.tile as tile
from concourse import mybir
from concourse.bass_utils import run_bass_kernel_spmd

B, S, H = 64, 2048, 512
NCORES = 8
BL = B // NCORES  # 8 batch rows per core
ST = 4            # number of s tiles
SW = S // ST      # 512 s per tile
GC = 4            # g (output-H) chunks of 128
HC = 4            # h (contraction) chunks of 128

F32 = mybir.dt.float32
F32R = mybir.dt.float32r

_cache = {}


def build(stage=99, nb=BL):
    A = mybir.AluOpType
    AF = mybir.ActivationFunctionType
    X = mybir.AxisListType.X

    nc = bacc.Bacc("TRN2", target_bir_lowering=False, debug=False,
                   num_devices=NCORES)

    keysT = nc.dram_tensor("keysT", [BL, H, S], F32R, kind="ExternalInput").ap()
    qT = nc.dram_tensor("qT", [H, BL], F32, kind="ExternalInput").ap()
    WwT = nc.dram_tensor("WwT", [H, H], F32, kind="ExternalInput").ap()
    UwT = nc.dram_tensor("UwT", [H, H], F32R, kind="ExternalInput").ap()
    cb = nc.dram_tensor("cb_rep", [128, GC * BL], F32, kind="ExternalInput").ap()
    vrep = nc.dram_tensor("v_rep", [128, H], F32R, kind="ExternalInput").ap()
    ctx_o = nc.dram_tensor("ctx_o", [BL, H], F32, kind="ExternalOutput").ap()
    attw_o = nc.dram_tensor("attw_o", [BL, S], F32, kind="ExternalOutput").ap()

    with tile.TileContext(nc) as tc, ExitStack() as ctx:
        const = ctx.enter_context(tc.tile_pool(name="const", bufs=1))
        kpool = ctx.enter_context(tc.tile_pool(name="keys", bufs=2 * ST))
        epool = ctx.enter_context(tc.tile_pool(name="energy", bufs=8))
        xpool = ctx.enter_context(tc.tile_pool(name="exp", bufs=2 * ST))
        spool = ctx.enter_context(tc.tile_pool(name="small", bufs=2))
        scr = ctx.enter_context(tc.tile_pool(name="scratch", bufs=2))
        upsum = ctx.enter_context(tc.tile_pool(name="upsum", bufs=2, space="PSUM"))
        lpsum = ctx.enter_context(tc.tile_pool(name="lpsum", bufs=5, space="PSUM"))
        wqps = ctx.enter_context(tc.tile_pool(name="wqps", bufs=1, space="PSUM"))

        # ---- constants / weights
        WwT_sb = const.tile([128, HC * H], F32)
        UwT_sb = const.tile([128, HC * H], F32R)
        qT_sb = const.tile([128, HC * BL], F32)
        for hc in range(HC):
            nc.sync.dma_start(WwT_sb[:, hc * H:(hc + 1) * H],
                              WwT[hc * 128:(hc + 1) * 128, :])
            nc.sync.dma_start(UwT_sb[:, hc * H:(hc + 1) * H],
                              UwT[hc * 128:(hc + 1) * 128, :])
            nc.sync.dma_start(qT_sb[:, hc * BL:(hc + 1) * BL],
                              qT[hc * 128:(hc + 1) * 128, :])
        cb_sb = const.tile([128, GC * BL], F32)
        nc.sync.dma_start(cb_sb[:], cb)
        v_sb = const.tile([128, H], F32R)
        nc.sync.dma_start(v_sb[:], vrep)

        # ---- biasT[g, (gc,b)] = (W_w @ q_b + W_b + U_b)[g]   (fp32 matmul)
        biasT = const.tile([128, GC * BL], F32)
        for gc in range(GC):
            wq = wqps.tile([128, BL], F32, tag="wq")
            for hc in range(HC):
                nc.tensor.matmul(
                    wq[:],
                    WwT_sb[:, hc * H + gc * 128: hc * H + (gc + 1) * 128],
                    qT_sb[:, hc * BL:(hc + 1) * BL],
                    start=(hc == 0), stop=(hc == HC - 1))
            nc.vector.tensor_add(biasT[:, gc * BL:(gc + 1) * BL], wq[:],
                                 cb_sb[:, gc * BL:(gc + 1) * BL])

        aw_sb = const.tile([BL, S], F32)
        rz_all = const.tile([BL, 1], F32)

        if stage >= 1:
            for b in range(nb):
                kts = []
                lgs = []
                mxp = spool.tile([128, ST], F32, tag="mxp")
                for st in range(ST):
                    kt = kpool.tile([128, HC * SW], F32R, tag="kt")
                    for hc in range(HC):
                        nc.sync.dma_start(
                            kt[:, hc * SW:(hc + 1) * SW],
                            keysT[b, hc * 128:(hc + 1) * 128,
                                  st * SW:(st + 1) * SW])
                    kts.append(kt)
                    ens = []
                    for gc in range(GC):
                        up = upsum.tile([128, SW], F32, tag="up")
                        for hc in range(HC):
                            nc.tensor.matmul(
                                up[:],
                                UwT_sb[:, hc * H + gc * 128: hc * H + (gc + 1) * 128],
                                kt[:, hc * SW:(hc + 1) * SW],
                                start=(hc == 0), stop=(hc == HC - 1))
                        en = epool.tile([128, SW], F32R, tag="en")
                        nc.scalar.activation(en[:], up[:], AF.Tanh,
                                             bias=biasT[:, gc * BL + b: gc * BL + b + 1],
                                             scale=1.0)
                        ens.append(en)
                    if stage < 2:
                        continue
                    lg = lpsum.tile([128, SW], F32, tag="lg")
                    for gc in range(GC):
                        nc.tensor.matmul(lg[:], v_sb[:, gc * 128:(gc + 1) * 128],
                                         ens[gc][:],
                                         start=(gc == 0), stop=(gc == GC - 1))
                    lgs.append(lg)
                    nc.vector.tensor_reduce(mxp[:, st:st + 1], lg[:], axis=X,
                                            op=A.max)

                if stage < 3:
                    continue
                # per-b epilogue: softmax + context
                mneg = spool.tile([128, 1], F32, tag="mneg")
                nc.vector.tensor_reduce(mneg[:], mxp[:], axis=X, op=A.max,
                                        negate=True)
                zp = spool.tile([128, ST], F32, tag="zp")
                exs = []
                for st in range(ST):
                    ex = xpool.tile([128, SW], F32, tag="ex")
                    nc.scalar.activation(ex[:], lgs[st][:], AF.Exp,
                                         bias=mneg[:, 0:1], scale=1.0,
                                         accum_out=zp[:, st:st + 1])
                    exs.append(ex)
                z = spool.tile([128, 1], F32, tag="z")
                nc.vector.tensor_reduce(z[:], zp[:], axis=X, op=A.add)
                rz = spool.tile([128, 1], F32, tag="rz")
                nc.vector.reciprocal(rz[:], z[:])

                if stage >= 4:
                    cparts = spool.tile([128, GC * ST], F32, tag="cparts")
                    for st in range(ST):
                        for hc in range(HC):
                            so = scr.tile([128, SW], F32, tag="so")
                            nc.vector.scalar_tensor_tensor(
                                out=so[:],
                                in0=kts[st][:, hc * SW:(hc + 1) * SW].bitcast(F32),
                                scalar=1.0,
                                in1=exs[st][:],
                                op0=A.mult, op1=A.mult,
                                accum_out=cparts[:, hc * ST + st: hc * ST + st + 1])
                    ctxv = spool.tile([128, HC], F32, tag="ctxv")
                    for hc in range(HC):
                        nc.vector.tensor_reduce(ctxv[:, hc:hc + 1],
                                                cparts[:, hc * ST:(hc + 1) * ST],
                                                axis=X, op=A.add)
                    ctxs = spool.tile([128, HC], F32, tag="ctxs")
                    nc.vector.tensor_scalar_mul(ctxs[:], ctxv[:], rz[:, 0:1])
                    for hc in range(HC):
                        nc.sync.dma_start(ctx_o[b, hc * 128:(hc + 1) * 128],
                                          ctxs[:, hc:hc + 1])
                if stage >= 5:
                    for st in range(ST):
                        nc.sync.dma_start(aw_sb[b:b + 1, st * SW:(st + 1) * SW],
                                          exs[st][0:1, :])
                    nc.sync.dma_start(rz_all[b:b + 1, 0:1], rz[0:1, 0:1])

        if stage >= 5:
            aw_out = const.tile([BL, S], F32)
            nc.scalar.mul(aw_out[:], aw_sb[:], rz_all[:, 0:1])
            nc.sync.dma_start(attw_o[:, :], aw_out[:])

    nc.compile()
    return nc


def _prep_shared(W_w, W_b, U_w, U_b, v_w):
    WwT = np.ascontiguousarray(W_w.T)
    UwT = np.ascontiguousarray(U_w.T)
    cbv = (U_b + W_b).reshape(GC, 128)  # [gc, p]
    cb = np.ascontiguousarray(
        np.repeat(cbv.T[:, :, None], BL, axis=2).reshape(128, GC * BL))
    vv = v_w.reshape(GC, 128)
    vr = np.ascontiguousarray(
        np.repeat(vv.T[:, :, None], 128, axis=2).reshape(128, H))
    return WwT, UwT, cb, vr


def kernel(query, keys, W_w, W_b, U_w, U_b, v_w, v_b):
    query = np.asarray(query, np.float32)
    keys = np.asarray(keys, np.float32)
    W_w = np.asarray(W_w, np.float32)
    W_b = np.asarray(W_b, np.float32)
    U_w = np.asarray(U_w, np.float32)
    U_b = np.asarray(U_b, np.float32)
    v_w = np.asarray(v_w, np.float32)

    if "nc" not in _cache:
        _cache["nc"] = build()
    nc = _cache["nc"]

    WwT, UwT, cb, vr = _prep_shared(W_w, W_b, U_w, U_b, v_w)
    in_maps = []
    for c in range(NCORES):
        sl = slice(c * BL, (c + 1) * BL)
        in_maps.append({
            "keysT": np.ascontiguousarray(keys[sl].transpose(0, 2, 1)),
            "qT": np.ascontiguousarray(query[sl].T),
            "WwT": WwT, "UwT": UwT, "cb_rep": cb, "v_rep": vr,
        })
    res = run_bass_kernel_spmd(nc, in_maps, core_ids=list(range(NCORES)))
    context = np.concatenate([res.results[c]["ctx_o"] for c in range(NCORES)], 0)
    attw = np.concatenate([res.results[c]["attw_o"] for c in range(NCORES)], 0)
    return context, attw
